# revision 33
# baseline (speedup 1.0000x reference)
"""Quantized 3x3 ConvBlock (NCHW, pad 1) on 8 Trainium2 NeuronCores.

Reference math (see problem):
  w_sum[o] = sum|W[o]|;  fw[o] = C1 / w_sum[o];  Wq = round(W * fw)
  fx = C2 / max|x|  (reference: global over all of x)
  xq = round(fx * x)
  y  = relu( conv(xq, Wq, pad=1) / (fx*fw[o]) + b[o] )

Key deviation (within the 2e-2 rel-err budget): the quantization scale fx
is computed PER IMAGE (fx_i = C2 / max|x_i|) instead of globally, and the
dequant uses 1/(fx_i*fw).  This is still an exact integer-quantized conv,
just with different rounding noise; measured rel err vs the reference is
~2.1e-3.  It removes the cross-core AllGather (+~40us of collective
latency) and the all-of-x barrier from the critical path entirely: image
0's conv can start as soon as image 0 is loaded (~25us), overlapping
image 1's load.

Implementation notes:
  - Data-parallel over batch: 2 images per core x 8 cores.
  - Single pass over x from HBM: each 16-row chunk is DMA'd once; the
    GpSimd (Pool) engine computes the chunk's abs-max ([128,2048]->[1,1])
    while the DVE copies it into the zero-padded fp16 [130x130] image.
    After the last chunk, fx_i is formed and the staged fp16 rows are
    quantized IN PLACE (DVE mult+magic-add -> Pool magic-sub), chunk by
    chunk, racing just ahead of the conv.
  - Conv = 9 shifted fp16 matmuls (contraction over in-channels = 128
    partitions) accumulated in PSUM per output tile of 4 rows x 128 cols.
    Quantized values are small integers (|xq| <= ~850, |Wq| <= ~110),
    exact in fp16; PSUM accumulates in fp32.
  - round() == round-half-even via the 1.5*2^23 magic-number add/sub.
  - x is staged to fp16 before rounding; this perturbs round(fx*x) by
    +-1 on a small fraction of elements (~0.2% output rel err).
  - y-out DMAs are issued from the Scalar engine (right after each
    activation) so the Sync engine's queue only carries x-in loads and
    image 1's load never queues behind conv output traffic.
"""

import numpy as np

N_CORES = 8
N_IMG, C_IN, H, W_DIM = 16, 128, 128, 128
C_OUT = 256
IMGS_PER_CORE = N_IMG // N_CORES  # 2
HP, WP = H + 2, W_DIM + 2  # padded 130x130
KK = 9
ROWS_PER_CHUNK = 16
CHUNKS_PER_IMG = H // ROWS_PER_CHUNK  # 8
CHUNK_ELEMS = ROWS_PER_CHUNK * W_DIM  # 2048
BLK_ROWS = 4
NBLK = H // BLK_ROWS  # 32

MAGIC = 12582912.0  # 1.5 * 2**23: add/sub rounds f32 to nearest-even integer

# Host-side scalar constants, computed in float64 exactly like the reference
_PRECISION = 2.0**24
_SF_CONST = 48.0
_NW = C_IN * KK  # 1152
_factor = np.sqrt(_PRECISION)
_sf = np.sqrt(_SF_CONST / _NW)
C1 = float(_factor / _sf - np.sqrt(_NW / 12.0) * 5.0)  # fw numerator
C2 = float(_factor * _sf - 0.5)  # fx numerator

_CACHE = {}
LAST_RESULTS = None  # BassKernelResults of the most recent run (for test.py)


def _build():
    import concourse.bacc as bacc
    import concourse.mybir as mybir
    import concourse.tile as tile
    from concourse.bass_isa import ReduceOp
    from concourse.masks import make_identity

    dt = mybir.dt
    AF = mybir.ActivationFunctionType
    ALU = mybir.AluOpType
    AX = mybir.AxisListType

    nc = bacc.Bacc(
        "TRN2",
        target_bir_lowering=False,
        debug=False,
        num_devices=N_CORES,
        name="convblock",
    )
    x_d = nc.dram_tensor(
        "x", [IMGS_PER_CORE, C_IN, H, W_DIM], dt.float32, kind="ExternalInput"
    )
    w_d = nc.dram_tensor("w", [C_OUT, _NW], dt.float32, kind="ExternalInput")
    b_d = nc.dram_tensor("b", [C_OUT, 1], dt.float32, kind="ExternalInput")
    y_d = nc.dram_tensor(
        "y", [IMGS_PER_CORE, C_OUT, H, W_DIM], dt.float32, kind="ExternalOutput"
    )

    with tile.TileContext(nc) as tc:
        with (
            tc.tile_pool(name="const", bufs=1) as constp,
            tc.tile_pool(name="wstage", bufs=1) as wstage,
            tc.tile_pool(name="xqpool", bufs=1) as xqpool,
            tc.tile_pool(name="stream", bufs=8) as stream,
            tc.tile_pool(name="tqpool", bufs=2) as tqpool,
            tc.tile_pool(name="outp", bufs=8) as outp,
            tc.tile_pool(name="psum", bufs=8, space="PSUM") as psum,
        ):
            # ---------------- weight prep (no dependency on x) ----------------
            identity = constp.tile([128, 128], dt.float16, name="identity")
            make_identity(nc, identity)

            fw_t = [None, None]
            bias_t = [None, None]
            wqT = [None] * (2 * KK)  # [128 in, 128 out] fp16, index = half*9+k

            def prep_half(h):
                """Load + quantize + transpose one 128-channel half of W.

                Emitted per half so that half 1's W DMA and PE transposes sit
                AFTER conv(0,0) in program order: they would otherwise block
                the first conv matmuls on the PE queue behind a W DMA that is
                itself queued behind image 0's x chunks."""
                wsb = wstage.tile(
                    [128, _NW], dt.float32, name=f"wsb{h}", tag="wsb"
                )
                # W rides the sync queue AHEAD of the x chunks: issued behind
                # them (or on another queue) it is starved by the 8MB x burst,
                # and wsum then head-of-line blocks every chunk reduce on DVE
                nc.sync.dma_start(wsb[:], w_d.ap()[h * 128 : (h + 1) * 128, :])
                wsum = constp.tile(
                    [128, 1], dt.float32, name=f"wsum{h}", tag=f"wsum{h}"
                )
                nc.vector.tensor_reduce(
                    wsum[:], wsb[:], axis=AX.X, op=ALU.add, apply_absolute_value=True
                )
                rws = constp.tile([128, 1], dt.float32, name=f"rws{h}", tag=f"rws{h}")
                nc.vector.reciprocal(rws[:], wsum[:])
                fw = constp.tile([128, 1], dt.float32, name=f"fw{h}", tag=f"fw{h}")
                nc.vector.tensor_scalar_mul(fw[:], rws[:], float(np.float32(C1)))
                fw_t[h] = fw

                # Wq = (W * fw + MAGIC) - MAGIC, stored fp16 in [out, in*9]
                # layout (both steps on ACT, which is idle this early)
                wqtmp = wstage.tile(
                    [128, _NW], dt.float32, name=f"wqtmp{h}", tag="wqtmp"
                )
                nc.scalar.activation(
                    wqtmp[:], wsb[:], AF.Copy, bias=MAGIC, scale=fw[:]
                )
                wqo = wstage.tile(
                    [128, _NW], dt.float16, name=f"wqo{h}", tag="wqo"
                )
                nc.scalar.activation(wqo[:], wqtmp[:], AF.Copy, bias=-MAGIC)

                # transpose each tap's [128 out, 128 in] to [128 in, 128 out]
                wqo3 = wqo.rearrange("p (i k) -> p i k", k=KK)
                for k in range(KK):
                    tp = psum.tile([128, 128], dt.float16, name="tp", tag="ps")
                    nc.tensor.transpose(tp[:], wqo3[:, :, k], identity[:])
                    wt = constp.tile(
                        [128, 128], dt.float16, name=f"wqT{h}_{k}", tag=f"wqT{h}_{k}"
                    )
                    nc.scalar.copy(wt[:], tp[:])
                    wqT[h * KK + k] = wt

                bt = constp.tile([128, 1], dt.float32, name=f"bias{h}", tag=f"bias{h}")
                nc.sync.dma_start(bt[:], b_d.ap()[h * 128 : (h + 1) * 128, :])
                bias_t[h] = bt

            # ---------------- per-image: load + max + quantize + conv --------
            x4 = x_d.ap()
            y4 = y_d.ap()
            v_t = [None, None]
            sc_t = [
                [[None, None], [None, None]],
                [[None, None], [None, None]],
            ]  # [img][half][h]

            def emit_scale(img, half, h, fx):
                # scale[o] = 1 / (fx * fw[o])
                den = constp.tile(
                    [128, 1], dt.float32,
                    name=f"den{img}_{half}_{h}", tag=f"den{img}_{half}_{h}",
                )
                nc.vector.tensor_mul(den[:], fx[:], fw_t[h][:])
                sc = constp.tile(
                    [128, 1], dt.float32,
                    name=f"sc{img}_{half}_{h}", tag=f"sc{img}_{half}_{h}",
                )
                nc.vector.reciprocal(sc[:], den[:])
                sc_t[img][half][h] = sc

            def stage(img):
                """Load image, take per-half maxes, quantize into two padded
                half-buffers.

                The image is split into TOP (output rows 0..63, padded rows
                0..65) and BOT (output rows 64..127, padded rows 64..129),
                each quantized with its own scale: fx_top only needs chunks
                0..4, so the top half's conv can start ~8us before the whole
                image has even arrived.  Both buffers carry the 2 overlap
                rows they read (quantized at their own scale), keeping every
                conv accumulation single-scale."""
                vT = xqpool.tile(
                    [128, 66 * WP], dt.float16, name=f"xqT{img}", tag=f"xqT{img}"
                )
                vB = xqpool.tile(
                    [128, 66 * WP], dt.float16, name=f"xqB{img}", tag=f"xqB{img}"
                )
                vT3 = vT.rearrange("p (h w) -> p h w", w=WP)  # padded rows 0..65
                vB3 = vB.rearrange("p (h w) -> p h w", w=WP)  # padded rows 64..129
                v_t[img] = (vT3, vB3)
                # borders: top row, bottom row, left/right columns
                nc.vector.memset(vT3[:, 0, :], 0.0)
                nc.vector.memset(vB3[:, 65, :], 0.0)
                nc.vector.memset(vT3[:, 1:66, 0], 0.0)
                nc.vector.memset(vT3[:, 1:66, WP - 1], 0.0)
                nc.vector.memset(vB3[:, 0:65, 0], 0.0)
                nc.vector.memset(vB3[:, 0:65, WP - 1], 0.0)

                # stream all 8 fp32 chunks in; they stay resident in the 8
                # stream slots until quantized into both half-buffers
                xcs = []
                for c in range(CHUNKS_PER_IMG):
                    r0 = c * ROWS_PER_CHUNK
                    xc = stream.tile(
                        [128, CHUNK_ELEMS], dt.float32, name="xc", tag="xc"
                    )
                    xcs.append(xc)
                    nc.sync.dma_start(xc[:], x4[img, :, r0 : r0 + ROWS_PER_CHUNK, :])

                HALF = CHUNK_ELEMS // 2
                cm = constp.tile(
                    [128, 10], dt.float32, name=f"cm{img}", tag=f"cm{img}"
                )  # cols: c0,c1,c2,c3,c4a,c4b,c5,c6,c7a,c7b

                def chunk_max(col, xc, lo, hi):
                    nc.vector.tensor_reduce(
                        cm[:, col : col + 1],
                        xc[:, lo:hi],
                        axis=AX.X,
                        op=ALU.max,
                        apply_absolute_value=True,
                    )

                def fx_of(lo_col, hi_col, name):
                    pm = constp.tile(
                        [128, 1], dt.float32, name=f"pm{name}", tag=f"pm{name}"
                    )
                    nc.vector.tensor_reduce(
                        pm[:], cm[:, lo_col:hi_col], axis=AX.X, op=ALU.max
                    )
                    xm = constp.tile(
                        [128, 1], dt.float32, name=f"xm{name}", tag=f"xm{name}"
                    )
                    nc.gpsimd.partition_all_reduce(xm[:], pm[:], 128, ReduceOp.max)
                    rx = constp.tile(
                        [128, 1], dt.float32, name=f"rx{name}", tag=f"rx{name}"
                    )
                    nc.vector.reciprocal(rx[:], xm[:])
                    fx = constp.tile(
                        [128, 1], dt.float32, name=f"fx{name}", tag=f"fx{name}"
                    )
                    nc.vector.tensor_scalar_mul(fx[:], rx[:], float(np.float32(C2)))
                    return fx

                def quant(src_ap, dst_rows_ap, fx, nelem):
                    tq = tqpool.tile([128, nelem], dt.float32, name="tq", tag="tq")
                    nc.vector.tensor_scalar(
                        tq[:], src_ap, fx[:], MAGIC, op0=ALU.mult, op1=ALU.add
                    )
                    nc.vector.tensor_scalar_sub(
                        dst_rows_ap,
                        tq.rearrange("p (h w) -> p h w", w=W_DIM),
                        MAGIC,
                    )

                # ---- TOP half: needs only chunks 0..4 (superset of x rows
                # 0..64); conv blocks 0..15 can start while chunks 5..7 land
                for c in range(4):
                    chunk_max(c, xcs[c], 0, CHUNK_ELEMS)
                chunk_max(4, xcs[4], 0, HALF)
                chunk_max(5, xcs[4], HALF, CHUNK_ELEMS)
                fxT = fx_of(0, 5, f"T{img}")  # chunks 0..4a: x rows 0..71
                emit_scale(img, 0, 0, fxT)
                emit_scale(img, 0, 1, fxT)
                # chunk 0 split in half so the first conv block fires earlier
                quant(xcs[0][:, 0:HALF], vT3[:, 1:9, 1 : 1 + W_DIM], fxT, HALF)
                quant(xcs[0][:, HALF:], vT3[:, 9:17, 1 : 1 + W_DIM], fxT, HALF)
                for c in range(1, 4):
                    quant(
                        xcs[c][:],
                        vT3[:, 1 + 16 * c : 17 + 16 * c, 1 : 1 + W_DIM],
                        fxT,
                        CHUNK_ELEMS,
                    )
                # x row 64 (chunk 4 row 0) -> vT padded row 65
                quant(xcs[4][:, 0:W_DIM], vT3[:, 65:66, 1 : 1 + W_DIM], fxT, W_DIM)

                # ---- BOT half: chunks 3..7 (superset of x rows 63..127)
                chunk_max(6, xcs[5], 0, CHUNK_ELEMS)
                chunk_max(7, xcs[6], 0, CHUNK_ELEMS)
                chunk_max(8, xcs[7], 0, HALF)
                chunk_max(9, xcs[7], HALF, CHUNK_ELEMS)
                fxB = fx_of(3, 10, f"B{img}")  # chunks 3..7: x rows 48..127
                emit_scale(img, 1, 0, fxB)
                emit_scale(img, 1, 1, fxB)
                # x row 63 (chunk 3 row 15) -> vB local row 0
                quant(
                    xcs[3][:, 15 * W_DIM : 16 * W_DIM],
                    vB3[:, 0:1, 1 : 1 + W_DIM],
                    fxB,
                    W_DIM,
                )
                for c in range(4, 8):
                    lr = 16 * c - 63
                    quant(
                        xcs[c][:],
                        vB3[:, lr : lr + 16, 1 : 1 + W_DIM],
                        fxB,
                        CHUNK_ELEMS,
                    )

            def conv_half(img, h):
                """9 accumulated matmuls per output tile of 4 rows x 128 cols."""
                vT3, vB3 = v_t[img]
                for blk in range(NBLK):
                    r0 = blk * BLK_ROWS
                    if blk < NBLK // 2:
                        v3, lr0, half = vT3, r0, 0
                    else:
                        v3, lr0, half = vB3, r0 - 64, 1
                    ps = psum.tile([128, 512], dt.float32, name="ps", tag="ps")
                    for k in range(KK):
                        kh, kw = divmod(k, 3)
                        rhs = v3[:, lr0 + kh : lr0 + kh + BLK_ROWS, kw : kw + W_DIM]
                        nc.tensor.matmul(
                            ps[:],
                            lhsT=wqT[h * KK + k][:],
                            rhs=rhs,
                            start=(k == 0),
                            stop=(k == KK - 1),
                        )
                    ot = outp.tile([128, 512], dt.float32, name="ot", tag="ot")
                    nc.scalar.activation(
                        ot[:],
                        ps[:],
                        AF.Relu,
                        bias=bias_t[h][:],
                        scale=sc_t[img][half][h][:],
                    )
                    nc.scalar.dma_start(
                        y4[img, h * 128 : (h + 1) * 128, r0 : r0 + BLK_ROWS, :],
                        ot.rearrange("p (r w) -> p r w", w=W_DIM),
                    )

            prep_half(0)
            prep_half(1)
            stage(0)
            conv_half(0, 0)
            conv_half(0, 1)
            stage(1)
            conv_half(1, 0)
            conv_half(1, 1)

    nc.compile()
    return nc


def kernel(x, W, b):
    global LAST_RESULTS
    from concourse.bass_utils import run_bass_kernel_spmd

    x = np.ascontiguousarray(np.asarray(x, dtype=np.float32))
    Wf = np.ascontiguousarray(np.asarray(W, dtype=np.float32).reshape(C_OUT, _NW))
    bf = np.ascontiguousarray(np.asarray(b, dtype=np.float32).reshape(C_OUT, 1))

    nc = _CACHE.get("nc")
    if nc is None:
        nc = _build()
        _CACHE["nc"] = nc

    in_maps = [
        {
            "x": x[c * IMGS_PER_CORE : (c + 1) * IMGS_PER_CORE],
            "w": Wf,
            "b": bf,
        }
        for c in range(N_CORES)
    ]
    res = run_bass_kernel_spmd(nc, in_maps, core_ids=list(range(N_CORES)))
    LAST_RESULTS = res
    y = np.concatenate(
        [res.results[c]["y"] for c in range(N_CORES)], axis=0
    )
    return y


# revision 40
# speedup vs baseline: 1.0842x; 1.0842x over previous
"""Quantized 3x3 ConvBlock (NCHW, pad 1) on 8 Trainium2 NeuronCores.

Reference math (see problem):
  w_sum[o] = sum|W[o]|;  fw[o] = C1 / w_sum[o];  Wq = round(W * fw)
  fx = C2 / max|x|  (reference: global over all of x)
  xq = round(fx * x)
  y  = relu( conv(xq, Wq, pad=1) / (fx*fw[o]) + b[o] )

Key deviation (within the 2e-2 rel-err budget): the quantization scale fx
is computed PER IMAGE (fx_i = C2 / max|x_i|) instead of globally, and the
dequant uses 1/(fx_i*fw).  This is still an exact integer-quantized conv,
just with different rounding noise; measured rel err vs the reference is
~2.1e-3.  It removes the cross-core AllGather (+~40us of collective
latency) and the all-of-x barrier from the critical path entirely: image
0's conv can start as soon as image 0 is loaded (~25us), overlapping
image 1's load.

Implementation notes:
  - Data-parallel over batch: 2 images per core x 8 cores.
  - Single pass over x from HBM: each 16-row chunk is DMA'd once; the
    GpSimd (Pool) engine computes the chunk's abs-max ([128,2048]->[1,1])
    while the DVE copies it into the zero-padded fp16 [130x130] image.
    After the last chunk, fx_i is formed and the staged fp16 rows are
    quantized IN PLACE (DVE mult+magic-add -> Pool magic-sub), chunk by
    chunk, racing just ahead of the conv.
  - Conv = 9 shifted fp16 matmuls (contraction over in-channels = 128
    partitions) accumulated in PSUM per output tile of 4 rows x 128 cols.
    Quantized values are small integers (|xq| <= ~850, |Wq| <= ~110),
    exact in fp16; PSUM accumulates in fp32.
  - round() == round-half-even via the 1.5*2^23 magic-number add/sub.
  - x is staged to fp16 before rounding; this perturbs round(fx*x) by
    +-1 on a small fraction of elements (~0.2% output rel err).
  - y-out DMAs are issued from the Scalar engine (right after each
    activation) so the Sync engine's queue only carries x-in loads and
    image 1's load never queues behind conv output traffic.
"""

import numpy as np

N_CORES = 8
N_IMG, C_IN, H, W_DIM = 16, 128, 128, 128
C_OUT = 256
IMGS_PER_CORE = N_IMG // N_CORES  # 2
HP, WP = H + 2, W_DIM + 2  # padded 130x130
KK = 9
ROWS_PER_CHUNK = 16
CHUNKS_PER_IMG = H // ROWS_PER_CHUNK  # 8
CHUNK_ELEMS = ROWS_PER_CHUNK * W_DIM  # 2048
BLK_ROWS = 4
NBLK = H // BLK_ROWS  # 32

MAGIC = 12582912.0  # 1.5 * 2**23: add/sub rounds f32 to nearest-even integer

# Host-side scalar constants, computed in float64 exactly like the reference
_PRECISION = 2.0**24
_SF_CONST = 48.0
_NW = C_IN * KK  # 1152
_factor = np.sqrt(_PRECISION)
_sf = np.sqrt(_SF_CONST / _NW)
C1 = float(_factor / _sf - np.sqrt(_NW / 12.0) * 5.0)  # fw numerator
C2 = float(_factor * _sf - 0.5)  # fx numerator

_CACHE = {}
LAST_RESULTS = None  # BassKernelResults of the most recent run (for test.py)


def _build():
    import concourse.bacc as bacc
    import concourse.mybir as mybir
    import concourse.tile as tile
    from concourse.bass_isa import ReduceOp
    from concourse.masks import make_identity

    dt = mybir.dt
    AF = mybir.ActivationFunctionType
    ALU = mybir.AluOpType
    AX = mybir.AxisListType

    nc = bacc.Bacc(
        "TRN2",
        target_bir_lowering=False,
        debug=False,
        num_devices=N_CORES,
        name="convblock",
    )
    x_d = nc.dram_tensor(
        "x", [IMGS_PER_CORE, C_IN, H, W_DIM], dt.float32, kind="ExternalInput"
    )
    w_d = nc.dram_tensor("w", [C_OUT, _NW], dt.float32, kind="ExternalInput")
    b_d = nc.dram_tensor("b", [C_OUT, 1], dt.float32, kind="ExternalInput")
    y_d = nc.dram_tensor(
        "y", [IMGS_PER_CORE, C_OUT, H, W_DIM], dt.float32, kind="ExternalOutput"
    )

    with tile.TileContext(nc) as tc:
        with (
            tc.tile_pool(name="const", bufs=1) as constp,
            tc.tile_pool(name="wstage", bufs=1) as wstage,
            tc.tile_pool(name="xqpool", bufs=1) as xqpool,
            tc.tile_pool(name="stream", bufs=8) as stream,
            tc.tile_pool(name="tqpool", bufs=2) as tqpool,
            tc.tile_pool(name="outp", bufs=8) as outp,
            tc.tile_pool(name="psum", bufs=8, space="PSUM") as psum,
        ):
            # ---------------- weight prep (no dependency on x) ----------------
            fw_t = [None, None]
            bias_t = [None, None]
            wqT = [None] * (2 * KK)  # [128 in, 128 out] fp16, index = half*9+k

            def prep_half(h):
                """Load + quantize + transpose one 128-channel half of W.

                Emitted per half so that half 1's W DMA and PE transposes sit
                AFTER conv(0,0) in program order: they would otherwise block
                the first conv matmuls on the PE queue behind a W DMA that is
                itself queued behind image 0's x chunks."""
                wsb = wstage.tile(
                    [128, _NW], dt.float32, name=f"wsb{h}", tag="wsb"
                )
                # W rides the sync queue AHEAD of the x chunks: issued behind
                # them (or on another queue) it is starved by the 8MB x burst,
                # and wsum then head-of-line blocks every chunk reduce on DVE
                nc.sync.dma_start(wsb[:], w_d.ap()[h * 128 : (h + 1) * 128, :])
                wsum = constp.tile(
                    [128, 1], dt.float32, name=f"wsum{h}", tag=f"wsum{h}"
                )
                nc.vector.tensor_reduce(
                    wsum[:], wsb[:], axis=AX.X, op=ALU.add, apply_absolute_value=True
                )
                rws = constp.tile([128, 1], dt.float32, name=f"rws{h}", tag=f"rws{h}")
                nc.vector.reciprocal(rws[:], wsum[:])
                fw = constp.tile([128, 1], dt.float32, name=f"fw{h}", tag=f"fw{h}")
                nc.vector.tensor_scalar_mul(fw[:], rws[:], float(np.float32(C1)))
                fw_t[h] = fw

                # Wq = (W * fw + MAGIC) - MAGIC, stored fp16 in [out, in*9]
                # layout (both steps on ACT, which is idle this early)
                wqtmp = wstage.tile(
                    [128, _NW], dt.float32, name=f"wqtmp{h}", tag="wqtmp"
                )
                nc.scalar.activation(
                    wqtmp[:], wsb[:], AF.Copy, bias=MAGIC, scale=fw[:]
                )
                # magic-sub writes through a (i,k)->(k,i) permuted AP so wqo
                # lands TAP-MAJOR: each tap's [out,in] block is contiguous,
                # as the xbar DMA transpose below requires
                wqo = wstage.tile(
                    [128, _NW], dt.float16, name=f"wqo{h}", tag="wqo"
                )
                nc.scalar.activation(
                    wqo.rearrange("p (k i) -> p i k", i=C_IN),
                    wqtmp.rearrange("p (i k) -> p i k", k=KK),
                    AF.Copy,
                    bias=-MAGIC,
                )

                # transpose each tap's [128 out, 128 in] to [128 in, 128 out]
                # via SBUF->SBUF xbar DMA transpose: runs on DMA hardware, so
                # neither the PE (transpose+psum) nor the DVE/ACT (psum->sbuf
                # copy) touches the weight path -- the DVE head then has only
                # the chunk reduces, and ACT stays clear of PSUM (the ACT-copy
                # variant reproducibly triggered a ~1.95GHz PE state).  Issued
                # from the otherwise-idle GpSimd (h0) / Scalar (h1) queues so
                # the sync queue's x burst is undisturbed.
                wqo3 = wqo.rearrange("p (k i) -> p k i", i=C_IN)
                for k in range(KK):
                    wt = constp.tile(
                        [128, 128], dt.float16, name=f"wqT{h}_{k}", tag=f"wqT{h}_{k}"
                    )
                    nc.scalar.dma_start_transpose(wt[:], wqo3[:, k, :])
                    wqT[h * KK + k] = wt

                bt = constp.tile([128, 1], dt.float32, name=f"bias{h}", tag=f"bias{h}")
                nc.sync.dma_start(bt[:], b_d.ap()[h * 128 : (h + 1) * 128, :])
                bias_t[h] = bt

            # ---------------- per-image: load + max + quantize + conv --------
            x4 = x_d.ap()
            y4 = y_d.ap()
            v_t = [None, None]
            sc_t = [
                [[None, None], [None, None]],
                [[None, None], [None, None]],
            ]  # [img][half][h]

            def emit_scale(img, half, h, fx):
                # scale[o] = 1 / (fx * fw[o])
                den = constp.tile(
                    [128, 1], dt.float32,
                    name=f"den{img}_{half}_{h}", tag=f"den{img}_{half}_{h}",
                )
                nc.vector.tensor_mul(den[:], fx[:], fw_t[h][:])
                sc = constp.tile(
                    [128, 1], dt.float32,
                    name=f"sc{img}_{half}_{h}", tag=f"sc{img}_{half}_{h}",
                )
                nc.vector.reciprocal(sc[:], den[:])
                sc_t[img][half][h] = sc

            def stage(img):
                """Load image, take per-half maxes, quantize into two padded
                half-buffers.

                The image is split into TOP (output rows 0..63, padded rows
                0..65) and BOT (output rows 64..127, padded rows 64..129),
                each quantized with its own scale: fx_top only needs chunks
                0..4, so the top half's conv can start ~8us before the whole
                image has even arrived.  Both buffers carry the 2 overlap
                rows they read (quantized at their own scale), keeping every
                conv accumulation single-scale."""
                vT = xqpool.tile(
                    [128, 66 * WP], dt.float16, name=f"xqT{img}", tag=f"xqT{img}"
                )
                vB = xqpool.tile(
                    [128, 66 * WP], dt.float16, name=f"xqB{img}", tag=f"xqB{img}"
                )
                vT3 = vT.rearrange("p (h w) -> p h w", w=WP)  # padded rows 0..65
                vB3 = vB.rearrange("p (h w) -> p h w", w=WP)  # padded rows 64..129
                v_t[img] = (vT3, vB3)
                # borders: top row, bottom row, left/right columns
                nc.vector.memset(vT3[:, 0, :], 0.0)
                nc.vector.memset(vB3[:, 65, :], 0.0)
                nc.vector.memset(vT3[:, 1:66, 0], 0.0)
                nc.vector.memset(vT3[:, 1:66, WP - 1], 0.0)
                nc.vector.memset(vB3[:, 0:65, 0], 0.0)
                nc.vector.memset(vB3[:, 0:65, WP - 1], 0.0)

                # stream all 8 fp32 chunks in; they stay resident in the 8
                # stream slots until quantized into both half-buffers
                xcs = []
                for c in range(CHUNKS_PER_IMG):
                    r0 = c * ROWS_PER_CHUNK
                    xc = stream.tile(
                        [128, CHUNK_ELEMS], dt.float32, name="xc", tag="xc"
                    )
                    xcs.append(xc)
                    nc.sync.dma_start(xc[:], x4[img, :, r0 : r0 + ROWS_PER_CHUNK, :])

                HALF = CHUNK_ELEMS // 2
                cm = constp.tile(
                    [128, 10], dt.float32, name=f"cm{img}", tag=f"cm{img}"
                )  # cols: c0,c1,c2,c3,c4a,c4b,c5,c6,c7a,c7b

                def chunk_max(col, xc, lo, hi):
                    nc.vector.tensor_reduce(
                        cm[:, col : col + 1],
                        xc[:, lo:hi],
                        axis=AX.X,
                        op=ALU.max,
                        apply_absolute_value=True,
                    )

                def fx_of(lo_col, hi_col, name):
                    pm = constp.tile(
                        [128, 1], dt.float32, name=f"pm{name}", tag=f"pm{name}"
                    )
                    nc.vector.tensor_reduce(
                        pm[:], cm[:, lo_col:hi_col], axis=AX.X, op=ALU.max
                    )
                    xm = constp.tile(
                        [128, 1], dt.float32, name=f"xm{name}", tag=f"xm{name}"
                    )
                    nc.gpsimd.partition_all_reduce(xm[:], pm[:], 128, ReduceOp.max)
                    rx = constp.tile(
                        [128, 1], dt.float32, name=f"rx{name}", tag=f"rx{name}"
                    )
                    nc.vector.reciprocal(rx[:], xm[:])
                    fx = constp.tile(
                        [128, 1], dt.float32, name=f"fx{name}", tag=f"fx{name}"
                    )
                    nc.vector.tensor_scalar_mul(fx[:], rx[:], float(np.float32(C2)))
                    return fx

                def quant(src_ap, dst_rows_ap, fx, nelem):
                    tq = tqpool.tile([128, nelem], dt.float32, name="tq", tag="tq")
                    nc.vector.tensor_scalar(
                        tq[:], src_ap, fx[:], MAGIC, op0=ALU.mult, op1=ALU.add
                    )
                    nc.vector.tensor_scalar_sub(
                        dst_rows_ap,
                        tq.rearrange("p (h w) -> p h w", w=W_DIM),
                        MAGIC,
                    )

                # ---- TOP half: needs only chunks 0..4 (superset of x rows
                # 0..64); conv blocks 0..15 can start while chunks 5..7 land
                for c in range(4):
                    chunk_max(c, xcs[c], 0, CHUNK_ELEMS)
                chunk_max(4, xcs[4], 0, HALF)
                chunk_max(5, xcs[4], HALF, CHUNK_ELEMS)
                fxT = fx_of(0, 5, f"T{img}")  # chunks 0..4a: x rows 0..71
                emit_scale(img, 0, 0, fxT)
                emit_scale(img, 0, 1, fxT)
                # chunk 0 split in half so the first conv block fires earlier
                quant(xcs[0][:, 0:HALF], vT3[:, 1:9, 1 : 1 + W_DIM], fxT, HALF)
                quant(xcs[0][:, HALF:], vT3[:, 9:17, 1 : 1 + W_DIM], fxT, HALF)
                for c in range(1, 4):
                    quant(
                        xcs[c][:],
                        vT3[:, 1 + 16 * c : 17 + 16 * c, 1 : 1 + W_DIM],
                        fxT,
                        CHUNK_ELEMS,
                    )
                # x row 64 (chunk 4 row 0) -> vT padded row 65
                quant(xcs[4][:, 0:W_DIM], vT3[:, 65:66, 1 : 1 + W_DIM], fxT, W_DIM)

                # ---- BOT half: chunks 3..7 (superset of x rows 63..127)
                chunk_max(6, xcs[5], 0, CHUNK_ELEMS)
                chunk_max(7, xcs[6], 0, CHUNK_ELEMS)
                chunk_max(8, xcs[7], 0, HALF)
                chunk_max(9, xcs[7], HALF, CHUNK_ELEMS)
                fxB = fx_of(3, 10, f"B{img}")  # chunks 3..7: x rows 48..127
                emit_scale(img, 1, 0, fxB)
                emit_scale(img, 1, 1, fxB)
                # x row 63 (chunk 3 row 15) -> vB local row 0
                quant(
                    xcs[3][:, 15 * W_DIM : 16 * W_DIM],
                    vB3[:, 0:1, 1 : 1 + W_DIM],
                    fxB,
                    W_DIM,
                )
                for c in range(4, 8):
                    lr = 16 * c - 63
                    quant(
                        xcs[c][:],
                        vB3[:, lr : lr + 16, 1 : 1 + W_DIM],
                        fxB,
                        CHUNK_ELEMS,
                    )

            def conv_half(img, h):
                """9 accumulated matmuls per output tile of 4 rows x 128 cols."""
                vT3, vB3 = v_t[img]
                for blk in range(NBLK):
                    r0 = blk * BLK_ROWS
                    if blk < NBLK // 2:
                        v3, lr0, half = vT3, r0, 0
                    else:
                        v3, lr0, half = vB3, r0 - 64, 1
                    ps = psum.tile([128, 512], dt.float32, name="ps", tag="ps")
                    for k in range(KK):
                        kh, kw = divmod(k, 3)
                        rhs = v3[:, lr0 + kh : lr0 + kh + BLK_ROWS, kw : kw + W_DIM]
                        nc.tensor.matmul(
                            ps[:],
                            lhsT=wqT[h * KK + k][:],
                            rhs=rhs,
                            start=(k == 0),
                            stop=(k == KK - 1),
                        )
                    ot = outp.tile([128, 512], dt.float32, name="ot", tag="ot")
                    nc.scalar.activation(
                        ot[:],
                        ps[:],
                        AF.Relu,
                        bias=bias_t[h][:],
                        scale=sc_t[img][half][h][:],
                    )
                    nc.scalar.dma_start(
                        y4[img, h * 128 : (h + 1) * 128, r0 : r0 + BLK_ROWS, :],
                        ot.rearrange("p (r w) -> p r w", w=W_DIM),
                    )

            prep_half(0)
            prep_half(1)
            stage(0)
            conv_half(0, 0)
            conv_half(0, 1)
            stage(1)
            conv_half(1, 0)
            conv_half(1, 1)

    nc.compile()
    return nc


def kernel(x, W, b):
    global LAST_RESULTS
    from concourse.bass_utils import run_bass_kernel_spmd

    x = np.ascontiguousarray(np.asarray(x, dtype=np.float32))
    Wf = np.ascontiguousarray(np.asarray(W, dtype=np.float32).reshape(C_OUT, _NW))
    bf = np.ascontiguousarray(np.asarray(b, dtype=np.float32).reshape(C_OUT, 1))

    nc = _CACHE.get("nc")
    if nc is None:
        nc = _build()
        _CACHE["nc"] = nc

    in_maps = [
        {
            "x": x[c * IMGS_PER_CORE : (c + 1) * IMGS_PER_CORE],
            "w": Wf,
            "b": bf,
        }
        for c in range(N_CORES)
    ]
    res = run_bass_kernel_spmd(nc, in_maps, core_ids=list(range(N_CORES)))
    LAST_RESULTS = res
    y = np.concatenate(
        [res.results[c]["y"] for c in range(N_CORES)], axis=0
    )
    return y


# revision 42
# speedup vs baseline: 1.1878x; 1.0956x over previous
"""Quantized 3x3 ConvBlock (NCHW, pad 1) on 8 Trainium2 NeuronCores.

Reference math (see problem):
  w_sum[o] = sum|W[o]|;  fw[o] = C1 / w_sum[o];  Wq = round(W * fw)
  fx = C2 / max|x|  (reference: global over all of x)
  xq = round(fx * x)
  y  = relu( conv(xq, Wq, pad=1) / (fx*fw[o]) + b[o] )

Key deviation (within the 2e-2 rel-err budget): the quantization scale fx
is computed PER IMAGE (fx_i = C2 / max|x_i|) instead of globally, and the
dequant uses 1/(fx_i*fw).  This is still an exact integer-quantized conv,
just with different rounding noise; measured rel err vs the reference is
~2.1e-3.  It removes the cross-core AllGather (+~40us of collective
latency) and the all-of-x barrier from the critical path entirely: image
0's conv can start as soon as image 0 is loaded (~25us), overlapping
image 1's load.

Implementation notes:
  - Data-parallel over batch: 2 images per core x 8 cores.
  - Single pass over x from HBM: each 16-row chunk is DMA'd once; the
    GpSimd (Pool) engine computes the chunk's abs-max ([128,2048]->[1,1])
    while the DVE copies it into the zero-padded fp16 [130x130] image.
    After the last chunk, fx_i is formed and the staged fp16 rows are
    quantized IN PLACE (DVE mult+magic-add -> Pool magic-sub), chunk by
    chunk, racing just ahead of the conv.
  - Conv = 9 shifted fp16 matmuls (contraction over in-channels = 128
    partitions) accumulated in PSUM per output tile of 4 rows x 128 cols.
    Quantized values are small integers (|xq| <= ~850, |Wq| <= ~110),
    exact in fp16; PSUM accumulates in fp32.
  - round() == round-half-even via the 1.5*2^23 magic-number add/sub.
  - x is staged to fp16 before rounding; this perturbs round(fx*x) by
    +-1 on a small fraction of elements (~0.2% output rel err).
  - y-out DMAs are issued from the Scalar engine (right after each
    activation) so the Sync engine's queue only carries x-in loads and
    image 1's load never queues behind conv output traffic.
"""

import numpy as np

N_CORES = 8
N_IMG, C_IN, H, W_DIM = 16, 128, 128, 128
C_OUT = 256
IMGS_PER_CORE = N_IMG // N_CORES  # 2
HP, WP = H + 2, W_DIM + 2  # padded 130x130
KK = 9
ROWS_PER_CHUNK = 16
CHUNKS_PER_IMG = H // ROWS_PER_CHUNK  # 8
CHUNK_ELEMS = ROWS_PER_CHUNK * W_DIM  # 2048
BLK_ROWS = 4
NBLK = H // BLK_ROWS  # 32

MAGIC = 12582912.0  # 1.5 * 2**23: add/sub rounds f32 to nearest-even integer

# Host-side scalar constants, computed in float64 exactly like the reference
_PRECISION = 2.0**24
_SF_CONST = 48.0
_NW = C_IN * KK  # 1152
_factor = np.sqrt(_PRECISION)
_sf = np.sqrt(_SF_CONST / _NW)
C1 = float(_factor / _sf - np.sqrt(_NW / 12.0) * 5.0)  # fw numerator
C2 = float(_factor * _sf - 0.5)  # fx numerator

_CACHE = {}
LAST_RESULTS = None  # BassKernelResults of the most recent run (for test.py)


def _build():
    import concourse.bacc as bacc
    import concourse.mybir as mybir
    import concourse.tile as tile
    from concourse.bass_isa import ReduceOp
    from concourse.masks import make_identity

    dt = mybir.dt
    AF = mybir.ActivationFunctionType
    ALU = mybir.AluOpType
    AX = mybir.AxisListType

    nc = bacc.Bacc(
        "TRN2",
        target_bir_lowering=False,
        debug=False,
        num_devices=N_CORES,
        name="convblock",
    )
    x_d = nc.dram_tensor(
        "x", [IMGS_PER_CORE, C_IN, H, W_DIM], dt.float32, kind="ExternalInput"
    )
    w_d = nc.dram_tensor("w", [C_OUT, _NW], dt.float32, kind="ExternalInput")
    b_d = nc.dram_tensor("b", [C_OUT, 1], dt.float32, kind="ExternalInput")
    y_d = nc.dram_tensor(
        "y", [IMGS_PER_CORE, C_OUT, H, W_DIM], dt.float32, kind="ExternalOutput"
    )

    with tile.TileContext(nc) as tc:
        with (
            tc.tile_pool(name="const", bufs=1) as constp,
            tc.tile_pool(name="wstage", bufs=1) as wstage,
            tc.tile_pool(name="xqpool", bufs=1) as xqpool,
            tc.tile_pool(name="stream", bufs=8) as stream,
            tc.tile_pool(name="tqpool", bufs=2) as tqpool,
            tc.tile_pool(name="outp", bufs=6) as outp,
            tc.tile_pool(name="psum", bufs=8, space="PSUM") as psum,
        ):
            # ---------------- weight prep (no dependency on x) ----------------
            identity = constp.tile([128, 128], dt.float16, name="identity")
            make_identity(nc, identity)

            fw_t = [None, None]
            bias_t = [None, None]
            wqT = [None] * (2 * KK)  # [128 in, 128 out] fp16, index = half*9+k

            def prep_half(h):
                """Load + quantize + transpose one 128-channel half of W.

                Emitted per half so that half 1's W DMA and PE transposes sit
                AFTER conv(0,0) in program order: they would otherwise block
                the first conv matmuls on the PE queue behind a W DMA that is
                itself queued behind image 0's x chunks."""
                wsb = wstage.tile(
                    [128, _NW], dt.float32, name=f"wsb{h}", tag=f"wsb{h}"
                )
                # W rides the sync queue AHEAD of the x chunks: issued behind
                # them (or on another queue) it is starved by the 8MB x burst,
                # and wsum then head-of-line blocks every chunk reduce on DVE
                nc.sync.dma_start(wsb[:], w_d.ap()[h * 128 : (h + 1) * 128, :])
                wsum = constp.tile(
                    [128, 1], dt.float32, name=f"wsum{h}", tag=f"wsum{h}"
                )
                nc.vector.tensor_reduce(
                    wsum[:], wsb[:], axis=AX.X, op=ALU.add, apply_absolute_value=True
                )
                rws = constp.tile([128, 1], dt.float32, name=f"rws{h}", tag=f"rws{h}")
                nc.vector.reciprocal(rws[:], wsum[:])
                fw = constp.tile([128, 1], dt.float32, name=f"fw{h}", tag=f"fw{h}")
                nc.vector.tensor_scalar_mul(fw[:], rws[:], float(np.float32(C1)))
                fw_t[h] = fw

                # Wq = (W * fw + MAGIC) - MAGIC, stored fp16 in [out, in*9]
                # layout (both steps on ACT, which is idle this early)
                wqtmp = wstage.tile(
                    [128, _NW], dt.float32, name=f"wqtmp{h}", tag=f"wqtmp{h}"
                )
                nc.scalar.activation(
                    wqtmp[:], wsb[:], AF.Copy, bias=MAGIC, scale=fw[:]
                )
                wqo = wstage.tile(
                    [128, _NW], dt.float16, name=f"wqo{h}", tag=f"wqo{h}"
                )
                nc.scalar.activation(wqo[:], wqtmp[:], AF.Copy, bias=-MAGIC)

                # transpose each tap's [128 out, 128 in] to [128 in, 128 out]
                wqo3 = wqo.rearrange("p (i k) -> p i k", k=KK)
                for k in range(KK):
                    tp = psum.tile([128, 128], dt.float16, name="tp", tag="ps")
                    nc.tensor.transpose(tp[:], wqo3[:, :, k], identity[:])
                    wt = constp.tile(
                        [128, 128], dt.float16, name=f"wqT{h}_{k}", tag=f"wqT{h}_{k}"
                    )
                    nc.vector.tensor_copy(wt[:], tp[:])
                    wqT[h * KK + k] = wt

                bt = constp.tile([128, 1], dt.float32, name=f"bias{h}", tag=f"bias{h}")
                nc.sync.dma_start(bt[:], b_d.ap()[h * 128 : (h + 1) * 128, :])
                bias_t[h] = bt

            # ---------------- per-image: load + max + quantize + conv --------
            x4 = x_d.ap()
            y4 = y_d.ap()
            v_t = [None, None]
            sc_t = [
                [[None, None], [None, None]],
                [[None, None], [None, None]],
            ]  # [img][half][h]

            def emit_scale(img, half, h, fx):
                # scale[o] = 1 / (fx * fw[o])
                den = constp.tile(
                    [128, 1], dt.float32,
                    name=f"den{img}_{half}_{h}", tag=f"den{img}_{half}_{h}",
                )
                nc.vector.tensor_mul(den[:], fx[:], fw_t[h][:])
                sc = constp.tile(
                    [128, 1], dt.float32,
                    name=f"sc{img}_{half}_{h}", tag=f"sc{img}_{half}_{h}",
                )
                nc.vector.reciprocal(sc[:], den[:])
                sc_t[img][half][h] = sc

            def stage(img):
                """Load image, take per-half maxes, quantize into two padded
                half-buffers.

                The image is split into TOP (output rows 0..63, padded rows
                0..65) and BOT (output rows 64..127, padded rows 64..129),
                each quantized with its own scale: fx_top only needs chunks
                0..4, so the top half's conv can start ~8us before the whole
                image has even arrived.  Both buffers carry the 2 overlap
                rows they read (quantized at their own scale), keeping every
                conv accumulation single-scale."""
                vT = xqpool.tile(
                    [128, 66 * WP], dt.float16, name=f"xqT{img}", tag=f"xqT{img}"
                )
                vB = xqpool.tile(
                    [128, 66 * WP], dt.float16, name=f"xqB{img}", tag=f"xqB{img}"
                )
                vT3 = vT.rearrange("p (h w) -> p h w", w=WP)  # padded rows 0..65
                vB3 = vB.rearrange("p (h w) -> p h w", w=WP)  # padded rows 64..129
                v_t[img] = (vT3, vB3)
                # borders: top row, bottom row, left/right columns
                nc.vector.memset(vT3[:, 0, :], 0.0)
                nc.vector.memset(vB3[:, 65, :], 0.0)
                nc.vector.memset(vT3[:, 1:66, 0], 0.0)
                nc.vector.memset(vT3[:, 1:66, WP - 1], 0.0)
                nc.vector.memset(vB3[:, 0:65, 0], 0.0)
                nc.vector.memset(vB3[:, 0:65, WP - 1], 0.0)

                # stream all 8 fp32 chunks in; they stay resident in the 8
                # stream slots until quantized into both half-buffers
                xcs = []
                for c in range(CHUNKS_PER_IMG):
                    r0 = c * ROWS_PER_CHUNK
                    xc = stream.tile(
                        [128, CHUNK_ELEMS], dt.float32, name="xc", tag="xc"
                    )
                    xcs.append(xc)
                    nc.sync.dma_start(xc[:], x4[img, :, r0 : r0 + ROWS_PER_CHUNK, :])

                HALF = CHUNK_ELEMS // 2
                cm = constp.tile(
                    [128, 10], dt.float32, name=f"cm{img}", tag=f"cm{img}"
                )  # cols: c0,c1,c2,c3,c4a,c4b,c5,c6,c7a,c7b

                def chunk_max(col, xc, lo, hi):
                    nc.vector.tensor_reduce(
                        cm[:, col : col + 1],
                        xc[:, lo:hi],
                        axis=AX.X,
                        op=ALU.max,
                        apply_absolute_value=True,
                    )

                def fx_of(lo_col, hi_col, name):
                    pm = constp.tile(
                        [128, 1], dt.float32, name=f"pm{name}", tag=f"pm{name}"
                    )
                    nc.vector.tensor_reduce(
                        pm[:], cm[:, lo_col:hi_col], axis=AX.X, op=ALU.max
                    )
                    xm = constp.tile(
                        [128, 1], dt.float32, name=f"xm{name}", tag=f"xm{name}"
                    )
                    nc.gpsimd.partition_all_reduce(xm[:], pm[:], 128, ReduceOp.max)
                    rx = constp.tile(
                        [128, 1], dt.float32, name=f"rx{name}", tag=f"rx{name}"
                    )
                    nc.vector.reciprocal(rx[:], xm[:])
                    fx = constp.tile(
                        [128, 1], dt.float32, name=f"fx{name}", tag=f"fx{name}"
                    )
                    nc.vector.tensor_scalar_mul(fx[:], rx[:], float(np.float32(C2)))
                    return fx

                def quant(src_ap, dst_rows_ap, fx, nelem):
                    tq = tqpool.tile([128, nelem], dt.float32, name="tq", tag="tq")
                    nc.vector.tensor_scalar(
                        tq[:], src_ap, fx[:], MAGIC, op0=ALU.mult, op1=ALU.add
                    )
                    nc.vector.tensor_scalar_sub(
                        dst_rows_ap,
                        tq.rearrange("p (h w) -> p h w", w=W_DIM),
                        MAGIC,
                    )

                # ---- TOP half: needs only chunks 0..4 (superset of x rows
                # 0..64); conv blocks 0..15 can start while chunks 5..7 land
                for c in range(4):
                    chunk_max(c, xcs[c], 0, CHUNK_ELEMS)
                chunk_max(4, xcs[4], 0, HALF)
                chunk_max(5, xcs[4], HALF, CHUNK_ELEMS)
                fxT = fx_of(0, 5, f"T{img}")  # chunks 0..4a: x rows 0..71
                emit_scale(img, 0, 0, fxT)
                emit_scale(img, 0, 1, fxT)
                # chunk 0 split in half so the first conv block fires earlier
                quant(xcs[0][:, 0:HALF], vT3[:, 1:9, 1 : 1 + W_DIM], fxT, HALF)
                quant(xcs[0][:, HALF:], vT3[:, 9:17, 1 : 1 + W_DIM], fxT, HALF)
                for c in range(1, 4):
                    quant(
                        xcs[c][:],
                        vT3[:, 1 + 16 * c : 17 + 16 * c, 1 : 1 + W_DIM],
                        fxT,
                        CHUNK_ELEMS,
                    )
                # x row 64 (chunk 4 row 0) -> vT padded row 65
                quant(xcs[4][:, 0:W_DIM], vT3[:, 65:66, 1 : 1 + W_DIM], fxT, W_DIM)

                # ---- BOT half: chunks 3..7 (superset of x rows 63..127)
                chunk_max(6, xcs[5], 0, CHUNK_ELEMS)
                chunk_max(7, xcs[6], 0, CHUNK_ELEMS)
                chunk_max(8, xcs[7], 0, HALF)
                chunk_max(9, xcs[7], HALF, CHUNK_ELEMS)
                fxB = fx_of(3, 10, f"B{img}")  # chunks 3..7: x rows 48..127
                emit_scale(img, 1, 0, fxB)
                emit_scale(img, 1, 1, fxB)
                # x row 63 (chunk 3 row 15) -> vB local row 0
                quant(
                    xcs[3][:, 15 * W_DIM : 16 * W_DIM],
                    vB3[:, 0:1, 1 : 1 + W_DIM],
                    fxB,
                    W_DIM,
                )
                for c in range(4, 8):
                    lr = 16 * c - 63
                    quant(
                        xcs[c][:],
                        vB3[:, lr : lr + 16, 1 : 1 + W_DIM],
                        fxB,
                        CHUNK_ELEMS,
                    )

            def conv_half(img, h):
                """9 accumulated matmuls per output tile of 4 rows x 128 cols."""
                vT3, vB3 = v_t[img]
                for blk in range(NBLK):
                    r0 = blk * BLK_ROWS
                    if blk < NBLK // 2:
                        v3, lr0, half = vT3, r0, 0
                    else:
                        v3, lr0, half = vB3, r0 - 64, 1
                    ps = psum.tile([128, 512], dt.float32, name="ps", tag="ps")
                    for k in range(KK):
                        kh, kw = divmod(k, 3)
                        rhs = v3[:, lr0 + kh : lr0 + kh + BLK_ROWS, kw : kw + W_DIM]
                        nc.tensor.matmul(
                            ps[:],
                            lhsT=wqT[h * KK + k][:],
                            rhs=rhs,
                            start=(k == 0),
                            stop=(k == KK - 1),
                        )
                    ot = outp.tile([128, 512], dt.float32, name="ot", tag="ot")
                    nc.scalar.activation(
                        ot[:],
                        ps[:],
                        AF.Relu,
                        bias=bias_t[h][:],
                        scale=sc_t[img][half][h][:],
                    )
                    nc.scalar.dma_start(
                        y4[img, h * 128 : (h + 1) * 128, r0 : r0 + BLK_ROWS, :],
                        ot.rearrange("p (r w) -> p r w", w=W_DIM),
                    )

            prep_half(0)
            prep_half(1)
            stage(0)
            conv_half(0, 0)
            conv_half(0, 1)
            stage(1)
            conv_half(1, 0)
            conv_half(1, 1)

    nc.compile()
    return nc


def kernel(x, W, b):
    global LAST_RESULTS
    from concourse.bass_utils import run_bass_kernel_spmd

    x = np.ascontiguousarray(np.asarray(x, dtype=np.float32))
    Wf = np.ascontiguousarray(np.asarray(W, dtype=np.float32).reshape(C_OUT, _NW))
    bf = np.ascontiguousarray(np.asarray(b, dtype=np.float32).reshape(C_OUT, 1))

    nc = _CACHE.get("nc")
    if nc is None:
        nc = _build()
        _CACHE["nc"] = nc

    in_maps = [
        {
            "x": x[c * IMGS_PER_CORE : (c + 1) * IMGS_PER_CORE],
            "w": Wf,
            "b": bf,
        }
        for c in range(N_CORES)
    ]
    res = run_bass_kernel_spmd(nc, in_maps, core_ids=list(range(N_CORES)))
    LAST_RESULTS = res
    y = np.concatenate(
        [res.results[c]["y"] for c in range(N_CORES)], axis=0
    )
    return y


# revision 43
# speedup vs baseline: 1.1886x; 1.0007x over previous
"""Quantized 3x3 ConvBlock (NCHW, pad 1) on 8 Trainium2 NeuronCores.

Reference math (see problem):
  w_sum[o] = sum|W[o]|;  fw[o] = C1 / w_sum[o];  Wq = round(W * fw)
  fx = C2 / max|x|  (reference: global over all of x)
  xq = round(fx * x)
  y  = relu( conv(xq, Wq, pad=1) / (fx*fw[o]) + b[o] )

Key deviation (within the 2e-2 rel-err budget): the quantization scale fx
is computed PER IMAGE (fx_i = C2 / max|x_i|) instead of globally, and the
dequant uses 1/(fx_i*fw).  This is still an exact integer-quantized conv,
just with different rounding noise; measured rel err vs the reference is
~2.1e-3.  It removes the cross-core AllGather (+~40us of collective
latency) and the all-of-x barrier from the critical path entirely: image
0's conv can start as soon as image 0 is loaded (~25us), overlapping
image 1's load.

Implementation notes:
  - Data-parallel over batch: 2 images per core x 8 cores.
  - Single pass over x from HBM: each 16-row chunk is DMA'd once; the
    GpSimd (Pool) engine computes the chunk's abs-max ([128,2048]->[1,1])
    while the DVE copies it into the zero-padded fp16 [130x130] image.
    After the last chunk, fx_i is formed and the staged fp16 rows are
    quantized IN PLACE (DVE mult+magic-add -> Pool magic-sub), chunk by
    chunk, racing just ahead of the conv.
  - Conv = 9 shifted fp16 matmuls (contraction over in-channels = 128
    partitions) accumulated in PSUM per output tile of 4 rows x 128 cols.
    Quantized values are small integers (|xq| <= ~850, |Wq| <= ~110),
    exact in fp16; PSUM accumulates in fp32.
  - round() == round-half-even via the 1.5*2^23 magic-number add/sub.
  - x is staged to fp16 before rounding; this perturbs round(fx*x) by
    +-1 on a small fraction of elements (~0.2% output rel err).
  - y-out DMAs are issued from the Scalar engine (right after each
    activation) so the Sync engine's queue only carries x-in loads and
    image 1's load never queues behind conv output traffic.
"""

import numpy as np

N_CORES = 8
N_IMG, C_IN, H, W_DIM = 16, 128, 128, 128
C_OUT = 256
IMGS_PER_CORE = N_IMG // N_CORES  # 2
HP, WP = H + 2, W_DIM + 2  # padded 130x130
KK = 9
ROWS_PER_CHUNK = 16
CHUNKS_PER_IMG = H // ROWS_PER_CHUNK  # 8
CHUNK_ELEMS = ROWS_PER_CHUNK * W_DIM  # 2048
BLK_ROWS = 4
NBLK = H // BLK_ROWS  # 32

MAGIC = 12582912.0  # 1.5 * 2**23: add/sub rounds f32 to nearest-even integer

# Host-side scalar constants, computed in float64 exactly like the reference
_PRECISION = 2.0**24
_SF_CONST = 48.0
_NW = C_IN * KK  # 1152
_factor = np.sqrt(_PRECISION)
_sf = np.sqrt(_SF_CONST / _NW)
C1 = float(_factor / _sf - np.sqrt(_NW / 12.0) * 5.0)  # fw numerator
C2 = float(_factor * _sf - 0.5)  # fx numerator

_CACHE = {}
LAST_RESULTS = None  # BassKernelResults of the most recent run (for test.py)


def _build():
    import concourse.bacc as bacc
    import concourse.mybir as mybir
    import concourse.tile as tile
    from concourse.bass_isa import ReduceOp
    from concourse.masks import make_identity

    dt = mybir.dt
    AF = mybir.ActivationFunctionType
    ALU = mybir.AluOpType
    AX = mybir.AxisListType

    nc = bacc.Bacc(
        "TRN2",
        target_bir_lowering=False,
        debug=False,
        num_devices=N_CORES,
        name="convblock",
    )
    x_d = nc.dram_tensor(
        "x", [IMGS_PER_CORE, C_IN, H, W_DIM], dt.float32, kind="ExternalInput"
    )
    w_d = nc.dram_tensor("w", [C_OUT, _NW], dt.float32, kind="ExternalInput")
    b_d = nc.dram_tensor("b", [C_OUT, 1], dt.float32, kind="ExternalInput")
    y_d = nc.dram_tensor(
        "y", [IMGS_PER_CORE, C_OUT, H, W_DIM], dt.float32, kind="ExternalOutput"
    )

    with tile.TileContext(nc) as tc:
        with (
            tc.tile_pool(name="const", bufs=1) as constp,
            tc.tile_pool(name="wstage", bufs=1) as wstage,
            tc.tile_pool(name="xqpool", bufs=1) as xqpool,
            tc.tile_pool(name="stream", bufs=8) as stream,
            tc.tile_pool(name="tqpool", bufs=2) as tqpool,
            tc.tile_pool(name="outp", bufs=6) as outp,
            tc.tile_pool(name="psum", bufs=8, space="PSUM") as psum,
        ):
            # ---------------- weight prep (no dependency on x) ----------------
            identity = constp.tile([128, 128], dt.float16, name="identity")
            make_identity(nc, identity)

            fw_t = [None, None]
            bias_t = [None, None]
            wqT = [None] * (2 * KK)  # [128 in, 128 out] fp16, index = half*9+k

            def prep_half(h):
                """Load + quantize + transpose one 128-channel half of W.

                Emitted per half so that half 1's W DMA and PE transposes sit
                AFTER conv(0,0) in program order: they would otherwise block
                the first conv matmuls on the PE queue behind a W DMA that is
                itself queued behind image 0's x chunks."""
                wsb = wstage.tile(
                    [128, _NW], dt.float32, name=f"wsb{h}", tag=f"wsb{h}"
                )
                # W rides the sync queue AHEAD of the x chunks: issued behind
                # them (or on another queue) it is starved by the 8MB x burst,
                # and wsum then head-of-line blocks every chunk reduce on DVE
                nc.sync.dma_start(wsb[:], w_d.ap()[h * 128 : (h + 1) * 128, :])
                wsum = constp.tile(
                    [128, 1], dt.float32, name=f"wsum{h}", tag=f"wsum{h}"
                )
                nc.vector.tensor_reduce(
                    wsum[:], wsb[:], axis=AX.X, op=ALU.add, apply_absolute_value=True
                )
                rws = constp.tile([128, 1], dt.float32, name=f"rws{h}", tag=f"rws{h}")
                nc.vector.reciprocal(rws[:], wsum[:])
                fw = constp.tile([128, 1], dt.float32, name=f"fw{h}", tag=f"fw{h}")
                nc.vector.tensor_scalar_mul(fw[:], rws[:], float(np.float32(C1)))
                fw_t[h] = fw

                # Wq = (W * fw + MAGIC) - MAGIC, stored fp16 in [out, in*9]
                # layout (both steps on ACT, which is idle this early)
                wqtmp = wstage.tile(
                    [128, _NW], dt.float32, name=f"wqtmp{h}", tag=f"wqtmp{h}"
                )
                nc.scalar.activation(
                    wqtmp[:], wsb[:], AF.Copy, bias=MAGIC, scale=fw[:]
                )
                wqo = wstage.tile(
                    [128, _NW], dt.float16, name=f"wqo{h}", tag=f"wqo{h}"
                )
                nc.scalar.activation(wqo[:], wqtmp[:], AF.Copy, bias=-MAGIC)

                # transpose each tap's [128 out, 128 in] to [128 in, 128 out]
                wqo3 = wqo.rearrange("p (i k) -> p i k", k=KK)
                for k in range(KK):
                    tp = psum.tile([128, 128], dt.float16, name="tp", tag="ps")
                    nc.tensor.transpose(tp[:], wqo3[:, :, k], identity[:])
                    wt = constp.tile(
                        [128, 128], dt.float16, name=f"wqT{h}_{k}", tag=f"wqT{h}_{k}"
                    )
                    nc.vector.tensor_copy(wt[:], tp[:])
                    wqT[h * KK + k] = wt

                bt = constp.tile([128, 1], dt.float32, name=f"bias{h}", tag=f"bias{h}")
                nc.sync.dma_start(bt[:], b_d.ap()[h * 128 : (h + 1) * 128, :])
                bias_t[h] = bt

            # ---------------- per-image: load + max + quantize + conv --------
            x4 = x_d.ap()
            y4 = y_d.ap()
            v_t = [None, None]
            sc_t = [
                [[None, None], [None, None]],
                [[None, None], [None, None]],
            ]  # [img][half][h]

            def emit_scale(img, half, h, fx):
                # scale[o] = 1 / (fx * fw[o])
                den = constp.tile(
                    [128, 1], dt.float32,
                    name=f"den{img}_{half}_{h}", tag=f"den{img}_{half}_{h}",
                )
                nc.vector.tensor_mul(den[:], fx[:], fw_t[h][:])
                sc = constp.tile(
                    [128, 1], dt.float32,
                    name=f"sc{img}_{half}_{h}", tag=f"sc{img}_{half}_{h}",
                )
                nc.vector.reciprocal(sc[:], den[:])
                sc_t[img][half][h] = sc

            def stage(img):
                """Load image, take per-half maxes, quantize into two padded
                half-buffers.

                The image is split into TOP (output rows 0..63, padded rows
                0..65) and BOT (output rows 64..127, padded rows 64..129),
                each quantized with its own scale: fx_top only needs chunks
                0..4, so the top half's conv can start ~8us before the whole
                image has even arrived.  Both buffers carry the 2 overlap
                rows they read (quantized at their own scale), keeping every
                conv accumulation single-scale."""
                vT = xqpool.tile(
                    [128, 62 * WP], dt.float16, name=f"xqT{img}", tag=f"xqT{img}"
                )
                vB = xqpool.tile(
                    [128, 70 * WP], dt.float16, name=f"xqB{img}", tag=f"xqB{img}"
                )
                vT3 = vT.rearrange("p (h w) -> p h w", w=WP)  # padded rows 0..65
                vB3 = vB.rearrange("p (h w) -> p h w", w=WP)  # padded rows 64..129
                v_t[img] = (vT3, vB3)
                # borders: top row, bottom row, left/right columns
                nc.vector.memset(vT3[:, 0, :], 0.0)
                nc.vector.memset(vB3[:, 69, :], 0.0)
                nc.vector.memset(vT3[:, 1:62, 0], 0.0)
                nc.vector.memset(vT3[:, 1:62, WP - 1], 0.0)
                nc.vector.memset(vB3[:, 0:69, 0], 0.0)
                nc.vector.memset(vB3[:, 0:69, WP - 1], 0.0)

                # stream all 8 fp32 chunks in; they stay resident in the 8
                # stream slots until quantized into both half-buffers
                xcs = []
                for c in range(CHUNKS_PER_IMG):
                    r0 = c * ROWS_PER_CHUNK
                    xc = stream.tile(
                        [128, CHUNK_ELEMS], dt.float32, name="xc", tag="xc"
                    )
                    xcs.append(xc)
                    nc.sync.dma_start(xc[:], x4[img, :, r0 : r0 + ROWS_PER_CHUNK, :])

                HALF = CHUNK_ELEMS // 2
                cm = constp.tile(
                    [128, 10], dt.float32, name=f"cm{img}", tag=f"cm{img}"
                )  # cols: c0,c1,c2,c3,c4a,c4b,c5,c6,c7a,c7b

                def chunk_max(col, xc, lo, hi):
                    nc.vector.tensor_reduce(
                        cm[:, col : col + 1],
                        xc[:, lo:hi],
                        axis=AX.X,
                        op=ALU.max,
                        apply_absolute_value=True,
                    )

                def fx_of(lo_col, hi_col, name):
                    pm = constp.tile(
                        [128, 1], dt.float32, name=f"pm{name}", tag=f"pm{name}"
                    )
                    nc.vector.tensor_reduce(
                        pm[:], cm[:, lo_col:hi_col], axis=AX.X, op=ALU.max
                    )
                    xm = constp.tile(
                        [128, 1], dt.float32, name=f"xm{name}", tag=f"xm{name}"
                    )
                    nc.gpsimd.partition_all_reduce(xm[:], pm[:], 128, ReduceOp.max)
                    rx = constp.tile(
                        [128, 1], dt.float32, name=f"rx{name}", tag=f"rx{name}"
                    )
                    nc.vector.reciprocal(rx[:], xm[:])
                    fx = constp.tile(
                        [128, 1], dt.float32, name=f"fx{name}", tag=f"fx{name}"
                    )
                    nc.vector.tensor_scalar_mul(fx[:], rx[:], float(np.float32(C2)))
                    return fx

                def quant(src_ap, dst_rows_ap, fx, nelem):
                    tq = tqpool.tile([128, nelem], dt.float32, name="tq", tag="tq")
                    nc.vector.tensor_scalar(
                        tq[:], src_ap, fx[:], MAGIC, op0=ALU.mult, op1=ALU.add
                    )
                    nc.vector.tensor_scalar_sub(
                        dst_rows_ap,
                        tq.rearrange("p (h w) -> p h w", w=W_DIM),
                        MAGIC,
                    )

                # ---- TOP half: output rows 0..59, padded rows 0..61 (x rows
                # 0..60).  The 60/68 split (vs 64/64) keeps fx_top's needs
                # inside chunks 0..3, so the top conv starts one full chunk
                # DMA (~3us) earlier; fx over chunks 0..3 is a superset max.
                for c in range(4):
                    chunk_max(c, xcs[c], 0, CHUNK_ELEMS)
                fxT = fx_of(0, 4, f"T{img}")  # chunks 0..3: x rows 0..63
                emit_scale(img, 0, 0, fxT)
                emit_scale(img, 0, 1, fxT)
                # chunk 0 split in half so the first conv block fires earlier
                quant(xcs[0][:, 0:HALF], vT3[:, 1:9, 1 : 1 + W_DIM], fxT, HALF)
                quant(xcs[0][:, HALF:], vT3[:, 9:17, 1 : 1 + W_DIM], fxT, HALF)
                for c in range(1, 3):
                    quant(
                        xcs[c][:],
                        vT3[:, 1 + 16 * c : 17 + 16 * c, 1 : 1 + W_DIM],
                        fxT,
                        CHUNK_ELEMS,
                    )
                # chunk 3 rows 0..12 (x rows 48..60) -> vT padded rows 49..61
                quant(
                    xcs[3][:, 0 : 13 * W_DIM],
                    vT3[:, 49:62, 1 : 1 + W_DIM],
                    fxT,
                    13 * W_DIM,
                )

                # ---- BOT half: output rows 60..127, padded rows 60..129
                # (x rows 59..127); fx over chunks 3..7 is a superset max
                chunk_max(4, xcs[4], 0, CHUNK_ELEMS)
                chunk_max(5, xcs[5], 0, CHUNK_ELEMS)
                chunk_max(6, xcs[6], 0, CHUNK_ELEMS)
                chunk_max(7, xcs[7], 0, HALF)
                chunk_max(8, xcs[7], HALF, CHUNK_ELEMS)
                fxB = fx_of(3, 9, f"B{img}")  # chunks 3..7: x rows 48..127
                emit_scale(img, 1, 0, fxB)
                emit_scale(img, 1, 1, fxB)
                # chunk 3 rows 11..15 (x rows 59..63) -> vB locals 0..4
                quant(
                    xcs[3][:, 11 * W_DIM : 16 * W_DIM],
                    vB3[:, 0:5, 1 : 1 + W_DIM],
                    fxB,
                    5 * W_DIM,
                )
                for c in range(4, 8):
                    lr = 16 * c - 59
                    quant(
                        xcs[c][:],
                        vB3[:, lr : lr + 16, 1 : 1 + W_DIM],
                        fxB,
                        CHUNK_ELEMS,
                    )

            def conv_half(img, h):
                """9 accumulated matmuls per output tile of 4 rows x 128 cols."""
                vT3, vB3 = v_t[img]
                for blk in range(NBLK):
                    r0 = blk * BLK_ROWS
                    if blk < 15:
                        v3, lr0, half = vT3, r0, 0
                    else:
                        v3, lr0, half = vB3, r0 - 60, 1
                    ps = psum.tile([128, 512], dt.float32, name="ps", tag="ps")
                    for k in range(KK):
                        kh, kw = divmod(k, 3)
                        rhs = v3[:, lr0 + kh : lr0 + kh + BLK_ROWS, kw : kw + W_DIM]
                        nc.tensor.matmul(
                            ps[:],
                            lhsT=wqT[h * KK + k][:],
                            rhs=rhs,
                            start=(k == 0),
                            stop=(k == KK - 1),
                        )
                    ot = outp.tile([128, 512], dt.float32, name="ot", tag="ot")
                    nc.scalar.activation(
                        ot[:],
                        ps[:],
                        AF.Relu,
                        bias=bias_t[h][:],
                        scale=sc_t[img][half][h][:],
                    )
                    nc.scalar.dma_start(
                        y4[img, h * 128 : (h + 1) * 128, r0 : r0 + BLK_ROWS, :],
                        ot.rearrange("p (r w) -> p r w", w=W_DIM),
                    )

            prep_half(0)
            prep_half(1)
            stage(0)
            conv_half(0, 0)
            conv_half(0, 1)
            stage(1)
            conv_half(1, 0)
            conv_half(1, 1)

    nc.compile()
    return nc


def kernel(x, W, b):
    global LAST_RESULTS
    from concourse.bass_utils import run_bass_kernel_spmd

    x = np.ascontiguousarray(np.asarray(x, dtype=np.float32))
    Wf = np.ascontiguousarray(np.asarray(W, dtype=np.float32).reshape(C_OUT, _NW))
    bf = np.ascontiguousarray(np.asarray(b, dtype=np.float32).reshape(C_OUT, 1))

    nc = _CACHE.get("nc")
    if nc is None:
        nc = _build()
        _CACHE["nc"] = nc

    in_maps = [
        {
            "x": x[c * IMGS_PER_CORE : (c + 1) * IMGS_PER_CORE],
            "w": Wf,
            "b": bf,
        }
        for c in range(N_CORES)
    ]
    res = run_bass_kernel_spmd(nc, in_maps, core_ids=list(range(N_CORES)))
    LAST_RESULTS = res
    y = np.concatenate(
        [res.results[c]["y"] for c in range(N_CORES)], axis=0
    )
    return y


# revision 44
# speedup vs baseline: 1.1901x; 1.0013x over previous
"""Quantized 3x3 ConvBlock (NCHW, pad 1) on 8 Trainium2 NeuronCores.

Reference math (see problem):
  w_sum[o] = sum|W[o]|;  fw[o] = C1 / w_sum[o];  Wq = round(W * fw)
  fx = C2 / max|x|  (reference: global over all of x)
  xq = round(fx * x)
  y  = relu( conv(xq, Wq, pad=1) / (fx*fw[o]) + b[o] )

Key deviation (within the 2e-2 rel-err budget): the quantization scale fx
is computed PER IMAGE (fx_i = C2 / max|x_i|) instead of globally, and the
dequant uses 1/(fx_i*fw).  This is still an exact integer-quantized conv,
just with different rounding noise; measured rel err vs the reference is
~2.1e-3.  It removes the cross-core AllGather (+~40us of collective
latency) and the all-of-x barrier from the critical path entirely: image
0's conv can start as soon as image 0 is loaded (~25us), overlapping
image 1's load.

Implementation notes:
  - Data-parallel over batch: 2 images per core x 8 cores.
  - Single pass over x from HBM: each 16-row chunk is DMA'd once; the
    GpSimd (Pool) engine computes the chunk's abs-max ([128,2048]->[1,1])
    while the DVE copies it into the zero-padded fp16 [130x130] image.
    After the last chunk, fx_i is formed and the staged fp16 rows are
    quantized IN PLACE (DVE mult+magic-add -> Pool magic-sub), chunk by
    chunk, racing just ahead of the conv.
  - Conv = 9 shifted fp16 matmuls (contraction over in-channels = 128
    partitions) accumulated in PSUM per output tile of 4 rows x 128 cols.
    Quantized values are small integers (|xq| <= ~850, |Wq| <= ~110),
    exact in fp16; PSUM accumulates in fp32.
  - round() == round-half-even via the 1.5*2^23 magic-number add/sub.
  - x is staged to fp16 before rounding; this perturbs round(fx*x) by
    +-1 on a small fraction of elements (~0.2% output rel err).
  - y-out DMAs are issued from the Scalar engine (right after each
    activation) so the Sync engine's queue only carries x-in loads and
    image 1's load never queues behind conv output traffic.
"""

import numpy as np

N_CORES = 8
N_IMG, C_IN, H, W_DIM = 16, 128, 128, 128
C_OUT = 256
IMGS_PER_CORE = N_IMG // N_CORES  # 2
HP, WP = H + 2, W_DIM + 2  # padded 130x130
KK = 9
ROWS_PER_CHUNK = 16
CHUNKS_PER_IMG = H // ROWS_PER_CHUNK  # 8
CHUNK_ELEMS = ROWS_PER_CHUNK * W_DIM  # 2048
BLK_ROWS = 4
NBLK = H // BLK_ROWS  # 32

MAGIC = 12582912.0  # 1.5 * 2**23: add/sub rounds f32 to nearest-even integer

# Host-side scalar constants, computed in float64 exactly like the reference
_PRECISION = 2.0**24
_SF_CONST = 48.0
_NW = C_IN * KK  # 1152
_factor = np.sqrt(_PRECISION)
_sf = np.sqrt(_SF_CONST / _NW)
C1 = float(_factor / _sf - np.sqrt(_NW / 12.0) * 5.0)  # fw numerator
C2 = float(_factor * _sf - 0.5)  # fx numerator

_CACHE = {}
LAST_RESULTS = None  # BassKernelResults of the most recent run (for test.py)


def _build():
    import concourse.bacc as bacc
    import concourse.mybir as mybir
    import concourse.tile as tile
    from concourse.bass_isa import ReduceOp
    from concourse.masks import make_identity

    dt = mybir.dt
    AF = mybir.ActivationFunctionType
    ALU = mybir.AluOpType
    AX = mybir.AxisListType

    nc = bacc.Bacc(
        "TRN2",
        target_bir_lowering=False,
        debug=False,
        num_devices=N_CORES,
        name="convblock",
    )
    x_d = nc.dram_tensor(
        "x", [IMGS_PER_CORE, C_IN, H, W_DIM], dt.float32, kind="ExternalInput"
    )
    w_d = nc.dram_tensor("w", [C_OUT, _NW], dt.float32, kind="ExternalInput")
    b_d = nc.dram_tensor("b", [C_OUT, 1], dt.float32, kind="ExternalInput")
    y_d = nc.dram_tensor(
        "y", [IMGS_PER_CORE, C_OUT, H, W_DIM], dt.float32, kind="ExternalOutput"
    )

    with tile.TileContext(nc) as tc:
        with (
            tc.tile_pool(name="const", bufs=1) as constp,
            tc.tile_pool(name="wstage", bufs=1) as wstage,
            tc.tile_pool(name="xqpool", bufs=1) as xqpool,
            tc.tile_pool(name="stream", bufs=8) as stream,
            tc.tile_pool(name="tqpool", bufs=2) as tqpool,
            tc.tile_pool(name="outp", bufs=6) as outp,
            tc.tile_pool(name="psum", bufs=8, space="PSUM") as psum,
        ):
            # ---------------- weight prep (no dependency on x) ----------------
            identity = constp.tile([128, 128], dt.float16, name="identity")
            make_identity(nc, identity)

            fw_t = [None, None]
            bias_t = [None, None]
            wqT = [None] * (2 * KK)  # [128 in, 128 out] fp16, index = half*9+k

            def prep_half(h):
                """Load + quantize + transpose one 128-channel half of W.

                Emitted per half so that half 1's W DMA and PE transposes sit
                AFTER conv(0,0) in program order: they would otherwise block
                the first conv matmuls on the PE queue behind a W DMA that is
                itself queued behind image 0's x chunks."""
                wsb = wstage.tile(
                    [128, _NW], dt.float32, name=f"wsb{h}", tag=f"wsb{h}"
                )
                # W rides the sync queue AHEAD of the x chunks: issued behind
                # them (or on another queue) it is starved by the 8MB x burst,
                # and wsum then head-of-line blocks every chunk reduce on DVE
                nc.sync.dma_start(wsb[:], w_d.ap()[h * 128 : (h + 1) * 128, :])
                wsum = constp.tile(
                    [128, 1], dt.float32, name=f"wsum{h}", tag=f"wsum{h}"
                )
                nc.vector.tensor_reduce(
                    wsum[:], wsb[:], axis=AX.X, op=ALU.add, apply_absolute_value=True
                )
                rws = constp.tile([128, 1], dt.float32, name=f"rws{h}", tag=f"rws{h}")
                nc.vector.reciprocal(rws[:], wsum[:])
                fw = constp.tile([128, 1], dt.float32, name=f"fw{h}", tag=f"fw{h}")
                nc.vector.tensor_scalar_mul(fw[:], rws[:], float(np.float32(C1)))
                fw_t[h] = fw

                # Wq = (W * fw + MAGIC) - MAGIC, stored fp16 in [out, in*9]
                # layout (both steps on ACT, which is idle this early)
                wqtmp = wstage.tile(
                    [128, _NW], dt.float32, name=f"wqtmp{h}", tag=f"wqtmp{h}"
                )
                nc.scalar.activation(
                    wqtmp[:], wsb[:], AF.Copy, bias=MAGIC, scale=fw[:]
                )
                wqo = wstage.tile(
                    [128, _NW], dt.float16, name=f"wqo{h}", tag=f"wqo{h}"
                )
                nc.scalar.activation(wqo[:], wqtmp[:], AF.Copy, bias=-MAGIC)

                # transpose each tap's [128 out, 128 in] to [128 in, 128 out]
                wqo3 = wqo.rearrange("p (i k) -> p i k", k=KK)
                for k in range(KK):
                    tp = psum.tile([128, 128], dt.float16, name="tp", tag="ps")
                    nc.tensor.transpose(tp[:], wqo3[:, :, k], identity[:])
                    wt = constp.tile(
                        [128, 128], dt.float16, name=f"wqT{h}_{k}", tag=f"wqT{h}_{k}"
                    )
                    nc.vector.tensor_copy(wt[:], tp[:])
                    wqT[h * KK + k] = wt

                bt = constp.tile([128, 1], dt.float32, name=f"bias{h}", tag=f"bias{h}")
                # b rides the idle scalar queue: it is only needed by the first
                # conv activation (~37us), and keeping it off the sync queue
                # lets the x chunks start ~1.3us earlier
                nc.scalar.dma_start(bt[:], b_d.ap()[h * 128 : (h + 1) * 128, :])
                bias_t[h] = bt

            # ---------------- per-image: load + max + quantize + conv --------
            x4 = x_d.ap()
            y4 = y_d.ap()
            v_t = [None, None]
            sc_t = [
                [[None, None], [None, None]],
                [[None, None], [None, None]],
            ]  # [img][half][h]

            def emit_scale(img, half, h, fx):
                # scale[o] = 1 / (fx * fw[o])
                den = constp.tile(
                    [128, 1], dt.float32,
                    name=f"den{img}_{half}_{h}", tag=f"den{img}_{half}_{h}",
                )
                nc.vector.tensor_mul(den[:], fx[:], fw_t[h][:])
                sc = constp.tile(
                    [128, 1], dt.float32,
                    name=f"sc{img}_{half}_{h}", tag=f"sc{img}_{half}_{h}",
                )
                nc.vector.reciprocal(sc[:], den[:])
                sc_t[img][half][h] = sc

            def stage(img):
                """Load image, take per-half maxes, quantize into two padded
                half-buffers.

                The image is split into TOP (output rows 0..63, padded rows
                0..65) and BOT (output rows 64..127, padded rows 64..129),
                each quantized with its own scale: fx_top only needs chunks
                0..4, so the top half's conv can start ~8us before the whole
                image has even arrived.  Both buffers carry the 2 overlap
                rows they read (quantized at their own scale), keeping every
                conv accumulation single-scale."""
                vT = xqpool.tile(
                    [128, 62 * WP], dt.float16, name=f"xqT{img}", tag=f"xqT{img}"
                )
                vB = xqpool.tile(
                    [128, 70 * WP], dt.float16, name=f"xqB{img}", tag=f"xqB{img}"
                )
                vT3 = vT.rearrange("p (h w) -> p h w", w=WP)  # padded rows 0..65
                vB3 = vB.rearrange("p (h w) -> p h w", w=WP)  # padded rows 64..129
                v_t[img] = (vT3, vB3)
                # borders: top row, bottom row, left/right columns
                nc.vector.memset(vT3[:, 0, :], 0.0)
                nc.vector.memset(vB3[:, 69, :], 0.0)
                nc.vector.memset(vT3[:, 1:62, 0], 0.0)
                nc.vector.memset(vT3[:, 1:62, WP - 1], 0.0)
                nc.vector.memset(vB3[:, 0:69, 0], 0.0)
                nc.vector.memset(vB3[:, 0:69, WP - 1], 0.0)

                # stream all 8 fp32 chunks in; they stay resident in the 8
                # stream slots until quantized into both half-buffers
                xcs = []
                for c in range(CHUNKS_PER_IMG):
                    r0 = c * ROWS_PER_CHUNK
                    xc = stream.tile(
                        [128, CHUNK_ELEMS], dt.float32, name="xc", tag="xc"
                    )
                    xcs.append(xc)
                    nc.sync.dma_start(xc[:], x4[img, :, r0 : r0 + ROWS_PER_CHUNK, :])

                HALF = CHUNK_ELEMS // 2
                cm = constp.tile(
                    [128, 10], dt.float32, name=f"cm{img}", tag=f"cm{img}"
                )  # cols: c0,c1,c2,c3,c4a,c4b,c5,c6,c7a,c7b

                def chunk_max(col, xc, lo, hi):
                    nc.vector.tensor_reduce(
                        cm[:, col : col + 1],
                        xc[:, lo:hi],
                        axis=AX.X,
                        op=ALU.max,
                        apply_absolute_value=True,
                    )

                def fx_of(lo_col, hi_col, name):
                    pm = constp.tile(
                        [128, 1], dt.float32, name=f"pm{name}", tag=f"pm{name}"
                    )
                    nc.vector.tensor_reduce(
                        pm[:], cm[:, lo_col:hi_col], axis=AX.X, op=ALU.max
                    )
                    xm = constp.tile(
                        [128, 1], dt.float32, name=f"xm{name}", tag=f"xm{name}"
                    )
                    nc.gpsimd.partition_all_reduce(xm[:], pm[:], 128, ReduceOp.max)
                    rx = constp.tile(
                        [128, 1], dt.float32, name=f"rx{name}", tag=f"rx{name}"
                    )
                    nc.vector.reciprocal(rx[:], xm[:])
                    fx = constp.tile(
                        [128, 1], dt.float32, name=f"fx{name}", tag=f"fx{name}"
                    )
                    nc.vector.tensor_scalar_mul(fx[:], rx[:], float(np.float32(C2)))
                    return fx

                def quant(src_ap, dst_rows_ap, fx, nelem):
                    tq = tqpool.tile([128, nelem], dt.float32, name="tq", tag="tq")
                    nc.vector.tensor_scalar(
                        tq[:], src_ap, fx[:], MAGIC, op0=ALU.mult, op1=ALU.add
                    )
                    nc.vector.tensor_scalar_sub(
                        dst_rows_ap,
                        tq.rearrange("p (h w) -> p h w", w=W_DIM),
                        MAGIC,
                    )

                # ---- TOP half: output rows 0..59, padded rows 0..61 (x rows
                # 0..60).  The 60/68 split (vs 64/64) keeps fx_top's needs
                # inside chunks 0..3, so the top conv starts one full chunk
                # DMA (~3us) earlier; fx over chunks 0..3 is a superset max.
                for c in range(4):
                    chunk_max(c, xcs[c], 0, CHUNK_ELEMS)
                fxT = fx_of(0, 4, f"T{img}")  # chunks 0..3: x rows 0..63
                emit_scale(img, 0, 0, fxT)
                emit_scale(img, 0, 1, fxT)
                # chunk 0 split in half so the first conv block fires earlier
                quant(xcs[0][:, 0:HALF], vT3[:, 1:9, 1 : 1 + W_DIM], fxT, HALF)
                quant(xcs[0][:, HALF:], vT3[:, 9:17, 1 : 1 + W_DIM], fxT, HALF)
                for c in range(1, 3):
                    quant(
                        xcs[c][:],
                        vT3[:, 1 + 16 * c : 17 + 16 * c, 1 : 1 + W_DIM],
                        fxT,
                        CHUNK_ELEMS,
                    )
                # chunk 3 rows 0..12 (x rows 48..60) -> vT padded rows 49..61
                quant(
                    xcs[3][:, 0 : 13 * W_DIM],
                    vT3[:, 49:62, 1 : 1 + W_DIM],
                    fxT,
                    13 * W_DIM,
                )

                # ---- BOT half: output rows 60..127, padded rows 60..129
                # (x rows 59..127); fx over chunks 3..7 is a superset max
                chunk_max(4, xcs[4], 0, CHUNK_ELEMS)
                chunk_max(5, xcs[5], 0, CHUNK_ELEMS)
                chunk_max(6, xcs[6], 0, CHUNK_ELEMS)
                chunk_max(7, xcs[7], 0, HALF)
                chunk_max(8, xcs[7], HALF, CHUNK_ELEMS)
                fxB = fx_of(3, 9, f"B{img}")  # chunks 3..7: x rows 48..127
                emit_scale(img, 1, 0, fxB)
                emit_scale(img, 1, 1, fxB)
                # chunk 3 rows 11..15 (x rows 59..63) -> vB locals 0..4
                quant(
                    xcs[3][:, 11 * W_DIM : 16 * W_DIM],
                    vB3[:, 0:5, 1 : 1 + W_DIM],
                    fxB,
                    5 * W_DIM,
                )
                for c in range(4, 8):
                    lr = 16 * c - 59
                    quant(
                        xcs[c][:],
                        vB3[:, lr : lr + 16, 1 : 1 + W_DIM],
                        fxB,
                        CHUNK_ELEMS,
                    )

            def conv_half(img, h):
                """9 accumulated matmuls per output tile of 4 rows x 128 cols."""
                vT3, vB3 = v_t[img]
                for blk in range(NBLK):
                    r0 = blk * BLK_ROWS
                    if blk < 15:
                        v3, lr0, half = vT3, r0, 0
                    else:
                        v3, lr0, half = vB3, r0 - 60, 1
                    ps = psum.tile([128, 512], dt.float32, name="ps", tag="ps")
                    for k in range(KK):
                        kh, kw = divmod(k, 3)
                        rhs = v3[:, lr0 + kh : lr0 + kh + BLK_ROWS, kw : kw + W_DIM]
                        nc.tensor.matmul(
                            ps[:],
                            lhsT=wqT[h * KK + k][:],
                            rhs=rhs,
                            start=(k == 0),
                            stop=(k == KK - 1),
                        )
                    ot = outp.tile([128, 512], dt.float32, name="ot", tag="ot")
                    nc.scalar.activation(
                        ot[:],
                        ps[:],
                        AF.Relu,
                        bias=bias_t[h][:],
                        scale=sc_t[img][half][h][:],
                    )
                    nc.scalar.dma_start(
                        y4[img, h * 128 : (h + 1) * 128, r0 : r0 + BLK_ROWS, :],
                        ot.rearrange("p (r w) -> p r w", w=W_DIM),
                    )

            prep_half(0)
            prep_half(1)
            stage(0)
            conv_half(0, 0)
            conv_half(0, 1)
            stage(1)
            conv_half(1, 0)
            conv_half(1, 1)

    nc.compile()
    return nc


def kernel(x, W, b):
    global LAST_RESULTS
    from concourse.bass_utils import run_bass_kernel_spmd

    x = np.ascontiguousarray(np.asarray(x, dtype=np.float32))
    Wf = np.ascontiguousarray(np.asarray(W, dtype=np.float32).reshape(C_OUT, _NW))
    bf = np.ascontiguousarray(np.asarray(b, dtype=np.float32).reshape(C_OUT, 1))

    nc = _CACHE.get("nc")
    if nc is None:
        nc = _build()
        _CACHE["nc"] = nc

    in_maps = [
        {
            "x": x[c * IMGS_PER_CORE : (c + 1) * IMGS_PER_CORE],
            "w": Wf,
            "b": bf,
        }
        for c in range(N_CORES)
    ]
    res = run_bass_kernel_spmd(nc, in_maps, core_ids=list(range(N_CORES)))
    LAST_RESULTS = res
    y = np.concatenate(
        [res.results[c]["y"] for c in range(N_CORES)], axis=0
    )
    return y
